# revision 26
# baseline (speedup 1.0000x reference)
"""Trainium2 Bass kernel for nn_Attention_7421703487529.

Multi-head attention, B=4 N=2048 C=512 H=8 D=64, fp32.
Sharding: 8 cores = 4 batches x 2 head-groups (4 heads each).
Each core computes, for its (batch b, head-group g):
    qkv^T = w_qkvT(g).T-slices @ x_b^T          -> Q^T,K^T,V^T  [768, 2048]
    V transposed on PE into [m,65] tiles with a ones column appended
    S^T = K @ Q^T (per head, keys on partitions), P^T = exp(S^T * scale)
    [O^T; denom] = [V|1].T @ P^T accumulated over key tiles in PSUM
    O_norm^T = O^T * (1/denom)  (DMA partition-broadcast of reciprocal row)
    yT_partial = w_proj[:, g-cols] @ O_all^T (+ b_proj on g==0 cores only)
Host: out[b] = (yT_partial[2b] + yT_partial[2b+1]).T
No collectives; cores are fully independent.

Perf notes:
- Attention matmuls in bf16 (fp32/f32r stream the moving operand at half
  rate); qkv + proj stay float32r for accuracy.
- The PE would idle ~25% waiting on ScalarE's exp in the attention loop,
  which makes the HAM activity monitor throttle the PE clock to 1.2 GHz.
  All PE work that is not order-critical (Q/K tiles for heads 2-3, V
  transposes for heads 2-3, first-half projection) is emitted as filler
  inside the attention loop to keep the PE dense and the clock at 2.4 GHz.
- query dim processed half-outer (cols 0-1023 then 1024-2047) so the
  half-0 projection overlaps half-1 attention.
"""

import sys

for _p in ("/opt/trn_rl_repo", "/root/.axon_site/_ro/trn_rl_repo"):
    if _p not in sys.path:
        sys.path.append(_p)

import ml_dtypes
import numpy as np

import concourse.bass as bass
import concourse.tile as tile
from concourse import bacc, mybir
from concourse.bass_utils import run_bass_kernel_spmd

F32 = mybir.dt.float32
F32R = mybir.dt.float32r
BF16 = mybir.dt.bfloat16

B, N, C = 4, 2048, 512
H, D = 8, 64
HG = 2              # head-groups (cores per batch)
HL = H // HG        # heads per core
CG = C // HG        # channels per group (= HL * D = 256)
SCALE = D ** -0.5
P = 128             # partitions
NT = N // P         # 16 key tiles per head
NH = 2              # query-dim halves (1024 cols each)
EXP = mybir.ActivationFunctionType.Exp


DEBUG_DUMPS = False


def _build_body(nc, xT, wqkvT, wpT, bias, identc, onesc, yT):
    from contextlib import ExitStack

    if DEBUG_DUMPS:
        dbg_qkvT = nc.dram_tensor("dbg_qkvT", [6, P, N], BF16,
                                  kind="ExternalOutput").ap()
        dbg_vaug = nc.dram_tensor("dbg_vaug", [HL, P, NT * 65], BF16,
                                  kind="ExternalOutput").ap()
        dbg_outT = nc.dram_tensor("dbg_outT", [2, P, N], F32R,
                                  kind="ExternalOutput").ap()

    with tile.TileContext(nc) as tc, ExitStack() as ctx:
        consts = ctx.enter_context(tc.tile_pool(name="consts", bufs=1))
        pT_pool = ctx.enter_context(tc.tile_pool(name="pT", bufs=3))
        rec_pool = ctx.enter_context(tc.tile_pool(name="rec", bufs=2))
        cpa_pool = ctx.enter_context(tc.tile_pool(name="cpa", bufs=3))
        bc_pool = ctx.enter_context(tc.tile_pool(name="bc", bufs=2))
        yT_pool = ctx.enter_context(tc.tile_pool(name="yTp", bufs=2))
        ps_pool = ctx.enter_context(tc.tile_pool(name="ps", bufs=2, space="PSUM"))
        aug_pool = ctx.enter_context(tc.tile_pool(name="aug", bufs=2, space="PSUM"))
        dram_pool = ctx.enter_context(tc.tile_pool(name="drp", bufs=2, space="DRAM"))

        # ---- inputs ----
        ident = consts.tile([P, 64], BF16, tag="ident")
        nc.sync.dma_start(out=ident, in_=identc[:, :])
        w_sb = [consts.tile([P, 3 * CG], BF16, tag=f"w{ct}", name=f"w{ct}")
                for ct in range(4)]
        for ct in range(4):
            nc.sync.dma_start(out=w_sb[ct], in_=wqkvT[ct * P:(ct + 1) * P, :])
        xT_sb = [consts.tile([P, N], BF16, tag=f"xT{ct}", name=f"xT{ct}")
                 for ct in range(4)]
        for hf in range(4):            # column-split loads: first MMs start sooner
            for ct in range(4):
                nc.sync.dma_start(
                    out=xT_sb[ct][:, hf * 512:(hf + 1) * 512],
                    in_=xT[ct * P:(ct + 1) * P, hf * 512:(hf + 1) * 512],
                )
        wp_sb = [consts.tile([P, C], BF16, tag=f"wp{ct}", name=f"wp{ct}")
                 for ct in range(2)]
        for ct in range(2):
            nc.sync.dma_start(out=wp_sb[ct], in_=wpT[ct * P:(ct + 1) * P, :])
        bias_sb = consts.tile([P, 4], F32, tag="bias")
        nc.sync.dma_start(
            out=bias_sb,
            in_=bass.AP(tensor=bias.tensor, offset=bias.offset, ap=[[1, P], [P, 4]]),
        )

        qkvT_sb = [consts.tile([P, N], BF16, tag=f"qkvT{jt}", name=f"qkvT{jt}")
                   for jt in range(6)]
        vaug_sb = [consts.tile([P, NT * 65], BF16, tag=f"vaug{l}", name=f"vaug{l}")
                   for l in range(HL)]
        vaug3 = [v.rearrange("p (i c) -> p i c", c=65) for v in vaug_sb]
        for l in range(HL):            # ones columns (denominator weights)
            nc.sync.dma_start(out=vaug3[l][:, :, 64], in_=onesc[:, :])
        outT_sb = [consts.tile([P, N], BF16, tag=f"outT{t}", name=f"outT{t}")
                   for t in range(2)]

        # ---- emission helpers ----
        def qkv_half(jt, half):
            # one [128,1024] psum tile: two 512-col accumulation chains + copy
            ps_q = ps_pool.tile([P, 1024], F32, tag="s", name="ps_q")
            for nb2 in range(2):
                col = half * 1024 + nb2 * 512
                for ct in range(4):
                    nc.tensor.matmul(
                        ps_q[:, nb2 * 512:(nb2 + 1) * 512],
                        lhsT=w_sb[ct][:, jt * P:(jt + 1) * P],
                        rhs=xT_sb[ct][:, col:col + 512],
                        start=(ct == 0),
                        stop=(ct == 3),
                    )
            nc.vector.tensor_copy(
                out=qkvT_sb[jt][:, half * 1024:(half + 1) * 1024], in_=ps_q[:, :]
            )

        def transpose_unit(l, i):
            rq = 64 * (l % 2)
            VT = qkvT_sb[4 + l // 2][rq:rq + 64, :]
            tp = aug_pool.tile([P, 64], BF16, tag="aug", name="tp")
            nc.tensor.transpose(tp[:, :], VT[:, i * P:(i + 1) * P],
                                ident[rq:rq + 64, :])
            nc.vector.tensor_copy(out=vaug3[l][:, i, 0:64], in_=tp[:, :])

        def proj_unit(ot, half):
            ps_y = aug_pool.tile([P, 1024], F32, tag="aug", name="ps_y")
            for nb2 in range(2):
                col = half * 1024 + nb2 * 512
                for ct in range(2):
                    nc.tensor.matmul(
                        ps_y[:, nb2 * 512:(nb2 + 1) * 512],
                        lhsT=wp_sb[ct][:, ot * P:(ot + 1) * P],
                        rhs=outT_sb[ct][:, col:col + 512],
                        start=(ct == 0),
                        stop=(ct == 1),
                    )
            yt = yT_pool.tile([P, 1024], F32, tag="yT", name="yt")
            nc.vector.tensor_scalar_add(
                out=yt[:, :], in0=ps_y[:, :], scalar1=bias_sb[:, ot:ot + 1]
            )
            nc.sync.dma_start(
                out=yT[ot * P:(ot + 1) * P, half * 1024:(half + 1) * 1024],
                in_=yt[:, :],
            )

        # ---- HAM warm-up: ~6us of back-to-back tiny matmuls on the ident
        # tile (arrives in the first DMA) promotes the PE clock to 2.4 GHz
        # while the big x/w DMAs are still in flight ----
        warm = ps_pool.tile([64, 64], F32, tag="s", name="warm")
        for _ in range(120):
            nc.tensor.matmul(warm[:, :], lhsT=ident[0:64, :], rhs=ident[0:64, :],
                             start=True, stop=True)

        # ---- minimal prologue: just what head 0 / half 0 needs ----
        for jt, half in ((4, 0), (4, 1), (0, 0), (2, 0)):
            qkv_half(jt, half)
        for i in range(NT):
            transpose_unit(0, i)

        # filler queue: independent PE work interleaved into the attention
        # loop to keep the tensor engine dense (HAM keeps the clock at 2.4GHz).
        # Each unit has an emission deadline (attention iteration index by
        # which its output must exist); surplus units are held back late so
        # the tail of the kernel stays dense too.
        fillers = []          # list of (deadline_iter, fn)

        def q(dl, fn, *a):
            fillers.append((dl, lambda: fn(*a)))

        q(4, qkv_half, 2, 1)               # K h0/h1, cols 1024+: S^T(h0,half0,i>=8)
        for i in range(NT):
            q(12 + i, transpose_unit, 1, i)   # vaug1: AV(h1, block 1)
        q(26, qkv_half, 1, 0)              # Q h2/h3 half0: S^T(h2, block 2)
        q(26, qkv_half, 3, 0)              # K h2/h3 half0
        q(24, qkv_half, 5, 0)              # V h2/h3 (feeds tr2/tr3)
        q(26, qkv_half, 5, 1)
        for i in range(NT):
            q(28 + i, transpose_unit, 2, i)   # vaug2: AV(h2, block 2)
        for i in range(NT):
            q(44 + i, transpose_unit, 3, i)   # vaug3: AV(h3, block 3)
        q(58, qkv_half, 0, 1)              # Q h0/h1 half1 (block 4)
        q(88, qkv_half, 1, 1)              # Q h2/h3 half1 (block 6)
        q(34, qkv_half, 3, 1)              # K h2/h3 cols 1024+: S^T(h2, block2, i>=8)
        fillers.sort(key=lambda u: u[0])

        # ---- attention: half outer so half-0 projection fills half-1 ----
        total_iters = NH * HL * NT
        done = 0
        for half in range(NH):
            for l in range(HL):
                rq = 64 * (l % 2)
                QT = qkvT_sb[l // 2][rq:rq + 64, :]
                KT = qkvT_sb[2 + l // 2][rq:rq + 64, :]
                vaug = vaug_sb[l]
                aug = aug_pool.tile([65, 1024], F32, tag="aug", name="aug")
                for i in range(NT):
                    stile = ps_pool.tile([P, 1024], F32, tag="s", name="stile")
                    for nb2 in range(2):
                        col = half * 1024 + nb2 * 512
                        nc.tensor.matmul(
                            stile[:, nb2 * 512:(nb2 + 1) * 512],
                            lhsT=KT[:, i * P:(i + 1) * P],
                            rhs=QT[:, col:col + 512],
                            start=True,
                            stop=True,
                        )
                    # pace fillers: pop whatever is near its deadline, plus
                    # a slow voluntary drain that keeps a reserve for the
                    # late blocks so the PE stays dense end-to-end
                    reserve = 6 if done < 96 else 0
                    npop = 0
                    while fillers and (fillers[0][0] <= done + 6
                                       or (len(fillers) > reserve
                                           and done % 2 == 0 and npop == 0)):
                        fillers.pop(0)[1]()
                        npop += 1
                        if npop >= 3:
                            break
                    done += 1
                    pT = pT_pool.tile([P, 1024], BF16, tag="pT")
                    nc.scalar.activation(out=pT[:, :], in_=stile[:, :], func=EXP,
                                         scale=float(SCALE))
                    for nb2 in range(2):
                        nc.tensor.matmul(
                            aug[:, nb2 * 512:(nb2 + 1) * 512],
                            lhsT=vaug[:, i * 65:i * 65 + 65],
                            rhs=pT[:, nb2 * 512:(nb2 + 1) * 512],
                            start=(i == 0),
                            stop=(i == NT - 1),
                        )

                # copy aug rows to SBUF immediately: frees the PSUM slot in
                # ~1us so the next block's accumulator/fillers aren't blocked
                cpa = cpa_pool.tile([65, 1024], F32, tag="cpa")
                nc.vector.tensor_copy(out=cpa[:, :], in_=aug[:, :])
                # denominator row spread over 16 partitions via DMA so the
                # DVE reciprocal runs 16-wide (0.13us instead of 6.5us)
                d16 = rec_pool.tile([16, 64], F32, tag="d16")
                nc.sync.dma_start(out=d16[:, :], in_=cpa[64:65, :])
                rec = rec_pool.tile([16, 64], F32, tag="rec")
                nc.vector.reciprocal(out=rec[:, :], in_=d16[:, :])
                # partition-broadcast via DRAM bounce (SBUF sources cannot
                # have stride-0 partition APs; DRAM sources can)
                rec_dr = dram_pool.tile([1, 1024], F32, tag="recd")
                nc.sync.dma_start(out=rec_dr[:, :], in_=rec[:, :])
                bc = bc_pool.tile([64, 1024], F32, tag="bc")
                nc.sync.dma_start(out=bc[:, :],
                                  in_=rec_dr[0:1, :].to_broadcast([64, 1024]))
                nc.vector.tensor_mul(
                    out=outT_sb[l // 2][rq:rq + 64, half * 1024:(half + 1) * 1024],
                    in0=cpa[0:64, :],
                    in1=bc[:, :],
                )

            if DEBUG_DUMPS and half == NH - 1:
                for jt in range(6):
                    nc.sync.dma_start(out=dbg_qkvT[jt], in_=qkvT_sb[jt][:, :])
                for l in range(HL):
                    nc.sync.dma_start(out=dbg_vaug[l], in_=vaug_sb[l][:, :])
                for t in range(2):
                    nc.sync.dma_start(out=dbg_outT[t], in_=outT_sb[t][:, :])

            # projection for this half; fills the next half's attention
            if half == 0:
                for ot in range(4):
                    fillers.append((104 + 6 * ot, lambda ot=ot: proj_unit(ot, 0)))
                fillers.sort(key=lambda u: u[0])
            else:
                for ot in range(4):
                    proj_unit(ot, 1)


def build_nc():
    nc = bacc.Bacc("TRN2", target_bir_lowering=False, debug=False, num_devices=8)
    xT = nc.dram_tensor("xT", [C, N], BF16, kind="ExternalInput").ap()
    wqkvT = nc.dram_tensor("wqkvT", [C, 3 * CG], BF16, kind="ExternalInput").ap()
    wpT = nc.dram_tensor("wpT", [CG, C], BF16, kind="ExternalInput").ap()
    bias = nc.dram_tensor("bias", [C], F32, kind="ExternalInput").ap()
    identc = nc.dram_tensor("identc", [P, 64], BF16, kind="ExternalInput").ap()
    onesc = nc.dram_tensor("onesc", [P, NT], BF16, kind="ExternalInput").ap()
    yT = nc.dram_tensor("yT", [C, N], F32, kind="ExternalOutput").ap()
    _build_body(nc, xT, wqkvT, wpT, bias, identc, onesc, yT)
    nc.compile()
    return nc


def make_in_maps(x, w_qkv, w_proj, b_proj):
    in_maps = []
    for core in range(8):
        b, g = core // 2, core % 2
        rows = np.concatenate([
            np.arange(CG * g, CG * g + CG),
            np.arange(C + CG * g, C + CG * g + CG),
            np.arange(2 * C + CG * g, 2 * C + CG * g + CG),
        ])
        in_maps.append({
            "xT": np.ascontiguousarray(x[b].T).astype(ml_dtypes.bfloat16),
            "wqkvT": np.ascontiguousarray(w_qkv[rows].T).astype(ml_dtypes.bfloat16),
            "wpT": np.ascontiguousarray(
                w_proj[:, CG * g:CG * (g + 1)].T).astype(ml_dtypes.bfloat16),
            "bias": (b_proj if g == 0 else np.zeros_like(b_proj)).astype(np.float32),
            "identc": np.vstack([np.eye(64)] * 2).astype(ml_dtypes.bfloat16),
            "onesc": np.ones((P, NT), ml_dtypes.bfloat16),
        })
    return in_maps


_NC = None


def _get_nc():
    global _NC
    if _NC is None:
        _NC = build_nc()
    return _NC


def run(x, w_qkv, w_proj, b_proj, trace=False, **kw):
    nc = _get_nc()
    in_maps = make_in_maps(
        np.asarray(x), np.asarray(w_qkv), np.asarray(w_proj), np.asarray(b_proj)
    )
    res = run_bass_kernel_spmd(nc, in_maps, list(range(8)), trace=trace, **kw)
    out = np.empty((B, N, C), np.float32)
    for b in range(B):
        out[b] = (res.results[2 * b]["yT"] + res.results[2 * b + 1]["yT"]).T
    return out, res


def kernel(x, w_qkv, w_proj, b_proj):
    out, _ = run(x, w_qkv, w_proj, b_proj, trace=False)
    return out
